# revision 13
# baseline (speedup 1.0000x reference)
"""Trainium2 Bass kernel for the SE-sweep DAG-RNN (nn_DAG_RNN_se).

Reference semantics (B=32, C=512, H=W=32):
    h[i,j] = relu(x[:,:,i,j] + (h[i-1,j] + h[i,j-1]) @ W_hh)     # [B, C]
    y[i,j] = h[i,j] @ W_yh + bias

Strategy:
  * Data-parallel over batch: 8 cores x 4 batch elements, zero communication.
  * Anti-diagonal wavefront inside a core: diagonal d holds n_d cells; all
    cells of a diagonal are batched into one set of matmuls.
  * State layout is transposed: h^T [C(4x128 partitions), n_d*B_local] so
    W_hh chunks are the stationary matmul operand; N = 4*n_d <= 128.
  * The kernel is LDWEIGHTS-issue bound (~75-100ns per matmul regardless of
    N<=128), so the x-injection identity matmuls of the baseline are gone:
    a custom DVE op RELU_ADD_SE computes h = relu(psum + x) in one Vector
    instruction, reading the W-only PSUM accumulation and the resident x.
  * PSUM is organised as chunk-PAIR banks (chunks 0,1 in bank A at column
    slots 0/256, chunks 2,3 in bank B), so one custom-op call produces a
    whole pair of h chunks, and the k-major matmul emission lets the pair-A
    bank close early (shorter cross-diagonal dependency cycle).
  * All y = h @ W_yh work (matmuls at N=512 where LDWEIGHTS hides under the
    moving-operand stream, bias adds, DMA-out) is emitted AFTER the whole
    recurrence at tail priorities: the Tile list scheduler then uses it
    purely as PE gap-filler and it can never head-of-line block the
    recurrence chain.
  * y is stored and DMA'd as fp16 (half the write traffic); the host
    upcasts. fp16 state + weights; PSUM accumulates fp32.

The full (unsharded) numpy contract is `kernel(**inputs)`; the Bass program
is built and compiled once and cached at module level.
"""

import sys

if "/opt/trn_rl_repo" not in sys.path:
    sys.path.insert(0, "/opt/trn_rl_repo")

import numpy as np

import concourse.bass as bass
import concourse.mybir as mybir
import concourse.tile as tile
from concourse import bacc
from concourse import bass_utils

# ---------------------------------------------------------------- constants
B, C, H, W = 32, 512, 32, 32
NCORES = 8
BL = B // NCORES            # local batch per core = 4
ND = H + W - 1              # 63 diagonals
CT = 4                      # channel chunks of 128
P = 128
SLOT = 512                  # per-chunk column slot inside a psum pair tile:
                            # one full bank per chunk, so the two interleaved
                            # accumulation groups of a pair never share a bank
                            # (same-bank interleaved groups corrupt PSUM)

F32 = mybir.dt.float32
F16 = mybir.dt.float16
ALU = mybir.AluOpType
ACTF = mybir.ActivationFunctionType

N_D = [min(d, H - 1) - max(0, d - (W - 1)) + 1 for d in range(ND)]
IMIN = [max(0, d - (W - 1)) for d in range(ND)]
OFFB = [0] * (ND + 1)
for _d in range(ND):
    OFFB[_d + 1] = OFFB[_d] + N_D[_d] * BL
TOT = OFFB[ND]              # 4096 columns per chunk row

# y output chunks (col0, width). The first 512 columns stay 128-wide: they
# become ready early (diag ~7+) and fill the chain-bound expanding-triangle
# bubbles; everything else is 512-wide (LDWEIGHTS fully hidden); the last
# 512 split in two so the forced serial tail after the final diagonal is
# half as long.
YCHUNKS = ([(i * 128, 128) for i in range(4)]
           + [(i * 512, 512) for i in range(1, 7)]
           + [(3584, 256), (3840, 256)])


def _register_relu_add():
    """Register the fused h = relu(psum + x) DVE op (idempotent)."""
    from concourse import dve_ops
    from concourse.dve_spec import Spec, Src0, Src1, relu, lower, _has_src1
    from concourse.dve_uop import DveOpSpec

    name = "RELU_ADD_SE"
    for op in dve_ops.OPS:
        if op.name == name:
            return op
    spec = Spec(
        body=relu(Src0 + Src1),
        reference=lambda in0, in1, s0, s1, imm2: np.maximum(
            in0.astype(np.float32) + in1.astype(np.float32), 0
        ),
    )
    row = max(dve_ops._SUB_OPCODE_FOR_NAME.values()) + 1
    shas = {}
    for ver in ("v3", "v4"):
        uops = lower(spec, ver=ver)
        shas[ver] = DveOpSpec(
            name=name, opcode=row, uops=uops, rd1_en=_has_src1(spec)
        ).sha(ver)
    op = dve_ops.DveOp(name, spec, subdim=False, uops_sha=shas)
    dve_ops._SUB_OPCODE_FOR_NAME[name] = row
    dve_ops.OPS.append(op)
    return op


def _build_program():
    relu_add = _register_relu_add()

    nc = bacc.Bacc("TRN2", target_bir_lowering=False, debug=False,
                   num_devices=NCORES)

    xs = nc.dram_tensor("xs", [P, CT * TOT], F16, kind="ExternalInput").ap()
    whh = nc.dram_tensor("whh", [C, C], F16, kind="ExternalInput").ap()
    wyh = nc.dram_tensor("wyh", [C, C], F16, kind="ExternalInput").ap()
    ident = nc.dram_tensor("ident", [P, P], F16, kind="ExternalInput").ap()
    biasp = nc.dram_tensor("biasp", [P, CT], F32, kind="ExternalInput").ap()
    y = nc.dram_tensor("y", [C, TOT], F16, kind="ExternalOutput").ap()

    with tile.TileContext(nc) as tc:
        with (
            tc.tile_pool(name="persist", bufs=1) as persist,
            tc.tile_pool(name="hspool", bufs=4) as hspool,
            tc.tile_pool(name="ypool", bufs=4) as ypool,
            tc.tile_pool(name="recps", bufs=2, space="PSUM") as recps,
            tc.tile_pool(name="yps", bufs=4, space="PSUM") as yps,
        ):
            # ---- resident tensors ----
            whh_sb = persist.tile([P, CT * C], F16, name="whh_sb")
            wyh_sb = persist.tile([P, CT * C], F16, name="wyh_sb")
            id_sb = persist.tile([P, P], F16, name="id_sb")
            bias_sb = persist.tile([P, CT], F32, name="bias_sb")
            # hidden state, chunk-major: chunk k occupies cols [k*TOT,(k+1)*TOT)
            hj = persist.tile([P, CT * TOT], F16, name="hj")
            # full input, resident: col q = CT*OFFB[d] + ct*(n_d*BL) + s*BL + b
            xsb = persist.tile([P, CT * TOT], F16, name="xsb")

            # Startup ordering matters: diag 1 needs only a tiny x prefix
            # and W_hh; W_yh/bias are not needed until the first y chunk.
            nc.sync.dma_start(xsb[:, 0:64], xs[:, 0:64])
            nc.sync.dma_start(id_sb[:], ident[:])
            for k in range(CT):
                nc.sync.dma_start(whh_sb[:, k * C:(k + 1) * C],
                                  whh[k * P:(k + 1) * P, :])
            nc.sync.dma_start(xsb[:, 64:512], xs[:, 64:512])
            nc.sync.dma_start(xsb[:, 512:2048], xs[:, 512:2048])
            NXD = 6
            w = (CT * TOT - 2048) // NXD
            for j in range(NXD):
                c0 = 2048 + j * w
                c1 = CT * TOT if j == NXD - 1 else c0 + w
                eng = nc.sync if j % 2 == 0 else nc.gpsimd
                eng.dma_start(xsb[:, c0:c1], xs[:, c0:c1])
            for k in range(CT):
                nc.gpsimd.dma_start(wyh_sb[:, k * C:(k + 1) * C],
                                    wyh[k * P:(k + 1) * P, :])
            nc.gpsimd.dma_start(bias_sb[:], biasp[:])

            def w_slice(wsb, k, ct):
                return wsb[:, k * C + ct * P: k * C + ct * P + P]

            def hjs(k, c0, wd):
                """h chunk-k cols [c0, c0+wd) as an AP."""
                return hj[:, k * TOT + c0: k * TOT + c0 + wd]

            def hj2(kbase, c0, wd):
                """strided pair view: chunks kbase,kbase+1, cols [c0,c0+wd)."""
                pair = hj[:, kbase * TOT:(kbase + 2) * TOT]
                return pair.rearrange("p (k q) -> p k q", k=2)[:, :, c0:c0 + wd]

            hs_prev = None     # list of 2 pair tiles [P, 2*N]
            for d in range(ND):
                n = N_D[d]
                N = n * BL
                x0 = CT * OFFB[d]

                if d + 1 < ND:
                    N2 = N_D[d + 1] * BL
                    hs_next = [hspool.tile([P, 2 * N2], F16, tag=f"hsp{pr}",
                                           name=f"hsp{pr}_{d + 1}")
                               for pr in range(2)]
                else:
                    hs_next = None

                if d == 0:
                    # h = relu(x): two pair-strided activations
                    for pr in range(2):
                        xv = xsb[:, x0 + 2 * pr * N: x0 + (2 * pr + 2) * N]
                        xv = xv.rearrange("p (k q) -> p k q", k=2)
                        nc.scalar.activation(hj2(2 * pr, OFFB[d], N), xv,
                                             ACTF.Relu)
                else:
                    # pair psum tiles: tile pr holds chunks 2pr, 2pr+1 at
                    # column slots 0/SLOT (one full bank per chunk — two
                    # interleaved accumulation groups must not share a bank).
                    psp = [recps.tile([P, 2 * SLOT], F32, tag="ps",
                                      name=f"ps{d}_{pr}")
                           for pr in range(2)]

                    def ps_out(g, width):
                        return psp[g // 2][:, (g % 2) * SLOT:
                                           (g % 2) * SLOT + width]

                    # x-injection for pair 0 via identity matmuls (their
                    # only input is resident x, so they fill PE bubbles
                    # while the chain waits); pair 1 gets x fused into the
                    # custom relu+add op instead.
                    for g in (0, 1):
                        nc.tensor.matmul(ps_out(g, N), lhsT=id_sb[:],
                                         rhs=xsb[:, x0 + g * N:
                                                 x0 + (g + 1) * N],
                                         start=True, stop=False)
                    # k-major: the k=0,1 matmuls need only pair-0 of the
                    # previous hsum (produced first); k=2,3 need pair-1.
                    for k in range(CT):
                        gs = range(CT) if k < CT - 1 else (0, 1)
                        for g in gs:
                            nc.tensor.matmul(
                                ps_out(g, N),
                                lhsT=w_slice(whh_sb, k, g),
                                rhs=hs_prev[k // 2][:, (k % 2) * N:
                                                    (k % 2 + 1) * N],
                                start=(k == 0 and g >= 2),
                                stop=(k == CT - 1))

                    # pair 0: h = relu(psum) on the Scalar engine (runs in
                    # parallel with the DVE work below)
                    psv0 = psp[0].rearrange("p (k q) -> p k q",
                                            k=2)[:, :, 0:N]
                    nc.scalar.activation(hj2(0, OFFB[d], N), psv0,
                                         ACTF.Relu)
                    for g in (2, 3):
                        nc.tensor.matmul(
                            ps_out(g, N),
                            lhsT=w_slice(whh_sb, CT - 1, g),
                            rhs=hs_prev[1][:, N:2 * N],
                            start=False, stop=True)
                    # pair 1: h = relu(psum + x), one fused DVE op; the
                    # pair-1 hsum STT follows it back-to-back on the DVE —
                    # this is the cross-diagonal critical cycle.
                    psv1 = psp[1].rearrange("p (k q) -> p k q",
                                            k=2)[:, :, 0:N]
                    xv = xsb[:, x0 + 2 * N: x0 + 4 * N]
                    xv = xv.rearrange("p (k q) -> p k q", k=2)
                    nc.vector._custom_dve(relu_add,
                                          out=hj2(2, OFFB[d], N),
                                          in0=psv1, in1=xv)

                # h_sum for diag d+1 from h (pair-strided shifted adds) —
                # pair 1 first (critical), then pair 0; boundary copies on
                # Pool (SBUF-only engine).
                if hs_next is not None:
                    for pr in (1, 0):
                        hs = hs_next[pr]
                        hsv = hs.rearrange("p (k q) -> p k q", k=2)
                        if d + 1 <= W - 1:
                            # expanding: n2 = n+1
                            if n > 1:
                                nc.vector.scalar_tensor_tensor(
                                    out=hsv[:, :, BL:n * BL],
                                    in0=hj2(2 * pr, OFFB[d], (n - 1) * BL),
                                    scalar=0.0, op0=ALU.bypass, op1=ALU.add,
                                    in1=hj2(2 * pr, OFFB[d] + BL,
                                            (n - 1) * BL))
                            nc.gpsimd.tensor_scalar_add(
                                hsv[:, :, 0:BL],
                                hj2(2 * pr, OFFB[d], BL), 0.0)
                            nc.gpsimd.tensor_scalar_add(
                                hsv[:, :, n * BL:(n + 1) * BL],
                                hj2(2 * pr, OFFB[d] + (n - 1) * BL, BL), 0.0)
                        else:
                            # contracting: n2 = n-1; hs[s] = h[s] + h[s+1]
                            nc.vector.scalar_tensor_tensor(
                                out=hsv[:, :, 0:(n - 1) * BL],
                                in0=hj2(2 * pr, OFFB[d], (n - 1) * BL),
                                scalar=0.0, op0=ALU.bypass, op1=ALU.add,
                                in1=hj2(2 * pr, OFFB[d] + BL, (n - 1) * BL))

                hs_prev = hs_next

            # ---- y = h @ W_yh + bias, emitted at tail priorities ----
            # The list scheduler pops these only when the recurrence isn't
            # ready: pure PE gap-filler, never blocks the chain by more
            # than one in-flight matmul.
            for ci_, (c0, wd) in enumerate(YCHUNKS):
                for ct in range(CT):
                    psy = yps.tile([P, 512], F32, tag="psy",
                                   name=f"psy{c0}_{ct}")
                    for k in range(CT):
                        nc.tensor.matmul(
                            psy[:, 0:wd],
                            lhsT=w_slice(wyh_sb, k, ct),
                            rhs=hjs(k, c0, wd),
                            start=(k == 0), stop=(k == CT - 1))
                    ysb = ypool.tile([P, 512], F16, tag="ysb",
                                     name=f"ysb{c0}_{ct}")
                    # alternate bias engine so neither FIFO queue's sem
                    # waits serialize the psy slot recycling
                    if (ci_ * CT + ct) % 2 == 0:
                        nc.vector.tensor_scalar_add(
                            ysb[:, 0:wd], psy[:, 0:wd],
                            bias_sb[:, ct:ct + 1])
                    else:
                        nc.scalar.activation(ysb[:, 0:wd], psy[:, 0:wd],
                                             ACTF.Identity,
                                             bias=bias_sb[:, ct:ct + 1],
                                             scale=1.0)
                    nc.sync.dma_start(
                        y[ct * P:(ct + 1) * P, c0:c0 + wd],
                        ysb[:, 0:wd])

    nc.compile()
    return nc


_CACHE = {}


def _get_program():
    if "nc" not in _CACHE:
        _CACHE["nc"] = _build_program()
    return _CACHE["nc"]


def _host_indices():
    """Precompute gather indices for host-side pre/post permutation."""
    if "idx" in _CACHE:
        return _CACHE["idx"]
    ct_of = np.empty(CT * TOT, dtype=np.int64)
    cell_of = np.empty(CT * TOT, dtype=np.int64)
    b_of = np.empty(CT * TOT, dtype=np.int64)
    cell_base = 0
    for d in range(ND):
        n = N_D[d]
        q0 = CT * OFFB[d]
        blk = n * BL
        for ct in range(CT):
            qs = q0 + ct * blk
            idx = np.arange(blk)
            ct_of[qs:qs + blk] = ct
            cell_of[qs:qs + blk] = cell_base + idx // BL
            b_of[qs:qs + blk] = idx % BL
        cell_base += n
    ci = np.empty(H * W, dtype=np.int64)
    cj = np.empty(H * W, dtype=np.int64)
    qcell = np.empty((H, W), dtype=np.int64)
    cell_base = 0
    for d in range(ND):
        for s in range(N_D[d]):
            i = IMIN[d] + s
            ci[cell_base] = i
            cj[cell_base] = d - i
            qcell[i, d - i] = OFFB[d] + s * BL
            cell_base += 1
    _CACHE["idx"] = (ct_of, cell_of, b_of, ci, cj, qcell)
    return _CACHE["idx"]


def make_in_maps(x, whh, wyh, b):
    ct_of, cell_of, b_of, ci, cj, qcell = _host_indices()
    whh16 = whh.astype(np.float16)
    wyh16 = wyh.astype(np.float16)
    id16 = np.eye(P, dtype=np.float16)
    biasp = np.ascontiguousarray(b.reshape(CT, P).T.astype(np.float32))
    xg = x[:, :, ci, cj]                             # [B, C, 1024]
    in_maps = []
    for c in range(NCORES):
        arr = xg[c * BL:(c + 1) * BL]                # [BL, C, 1024]
        arr3 = arr.reshape(BL, CT, P, H * W).transpose(2, 1, 3, 0)
        xs_core = np.ascontiguousarray(
            arr3[np.arange(P)[:, None], ct_of[None, :], cell_of[None, :],
                 b_of[None, :]].astype(np.float16))
        in_maps.append({"xs": xs_core, "whh": whh16, "wyh": wyh16,
                        "ident": id16, "biasp": biasp})
    return in_maps


def kernel(input, weight_hh, weight_yh, bias):
    x = np.ascontiguousarray(np.asarray(input, dtype=np.float32))
    whh = np.asarray(weight_hh, dtype=np.float32)
    wyh = np.asarray(weight_yh, dtype=np.float32)
    b = np.asarray(bias, dtype=np.float32)

    nc = _get_program()
    in_maps = make_in_maps(x, whh, wyh, b)
    res = bass_utils.run_bass_kernel_spmd(nc, in_maps,
                                          core_ids=list(range(NCORES)))

    _, _, _, _, _, qcell = _host_indices()
    out = np.empty((B, C, H, W), dtype=np.float32)
    qidx = qcell[None, :, :] + np.arange(BL)[:, None, None]
    for c in range(NCORES):
        ydev = res.results[c]["y"]                   # [512, 4096] f16
        out[c * BL:(c + 1) * BL] = (
            ydev[:, qidx].transpose(1, 0, 2, 3).astype(np.float32))
    return out
